# revision 2
# baseline (speedup 1.0000x reference)
"""Trainium2 Bass kernel for nn_ModelRQuery_5806795784426.

Strategy (data-parallel over bags, 8 cores x 64 bags):
  - node_weight (cosine-sim softmax) is computed with the exact same eager
    jax ops as the reference, so the Huffman merge schedule derived from it
    is bit-faithful to the reference's argmin decisions on this backend.
  - The Huffman weight evolution is replayed on host (pure IEEE f32 adds on
    identical bits -> identical schedule), producing per-bag merge pairs.
  - Per bag, the 63 merges are list-scheduled into R pair-rounds (2
    independent merges per round -> M=128 rows, full PE array) with
    children >= 3 rounds earlier (dist-3), so every round's gather is
    covered by a full round of PE work and the PE never waits on the
    scatter->gather chain.  Each bag's root merge is pinned to the final
    round, slot 0.
  - tanh is applied at PRODUCTION: DRAM feats rows hold tanh'd bf16
    features (leaves host-pre-tanh'd).
  - v2: ALL transposes are off the PE.  The gather is dma_gather
    (transpose=True): it both gathers the 2x128 child rows by index and
    writes them transposed (feature-major) into SBUF in one SWDGE
    instruction.  The FC1->FC2 h-transpose uses the SBUF-source variant of
    the same instruction.  The PE runs ONLY the FC1/FC2 fp8 DoubleRow
    matmuls, software-pipelined as [..., FC1_{r+1}, FC2_r, ...] so the
    tanh/transpose/gather latency of round r hides under FC1_{r+1}.
  - Final: raw f32 root features are DMA'd out; the tiny scores matmul
    (B,1024)@(1024,53) + sigmoid run on host.
"""

import numpy as np

NB = 64      # bags per core
NN = 64      # nodes (leaves) per bag
D = 1024
NSTEP = NN - 1
NCORES = 8
DIST = 3     # min round separation between child production and consumption

_PROG = {}


def _build_program(R, SL, zero_bias):
    key = (R, SL, zero_bias, "v2")
    if key in _PROG:
        return _PROG[key]
    import concourse.bass as bass
    import concourse.bacc as bacc
    import concourse.tile as tile

    mybir = bass.mybir
    f32 = mybir.dt.float32
    bf16 = mybir.dt.bfloat16
    f8 = mybir.dt.float8e4
    i16 = mybir.dt.int16
    TANH = mybir.ActivationFunctionType.Tanh
    ADD = mybir.AluOpType.add
    DR = mybir.MatmulPerfMode.DoubleRow

    nc = bacc.Bacc(None, target_bir_lowering=False, num_swdge_queues=3)
    # feats rows: bag*SL + slot, tanh'd bf16; slots 0..63 = leaves
    # (host-prefilled tanh(rep)), slot 64+2r+s = tanh(output) of round r
    # in-round slot s.
    feats_d = nc.dram_tensor("feats", [NB * SL, D], bf16, kind="ExternalInput")
    w1t_d = nc.dram_tensor("w1t", [2 * D, D], f8, kind="ExternalInput")
    w2t_d = nc.dram_tensor("w2t", [D, D], f8, kind="ExternalInput")
    b1b_d = nc.dram_tensor("b1b", [128, D], f32, kind="ExternalInput")
    b2b_d = nc.dram_tensor("b2b", [128, D], f32, kind="ExternalInput")
    # int16 gather indices: for round q, op h, the 128 gathered rows are at
    # gidx[:, 16q+8h : 16q+8h+8] in the SWDGE "wrapped-16" layout
    # (position n -> [n%16, n//16], replicated across partition groups).
    gidx_d = nc.dram_tensor("gidx", [128, 16 * R], i16, kind="ExternalInput")
    iota_d = nc.dram_tensor("iota", [128, 8], i16, kind="ExternalInput")
    out_d = nc.dram_tensor("out", [NB, D], f32, kind="ExternalOutput")

    with tile.TileContext(nc) as tc:
        with tc.tile_pool(name="const", bufs=1) as cp, \
             tc.tile_pool(name="xb", bufs=3) as xbp, \
             tc.tile_pool(name="xq", bufs=2) as xqp, \
             tc.tile_pool(name="hp", bufs=2) as hp, \
             tc.tile_pool(name="fp", bufs=2) as fp, \
             tc.tile_pool(name="mmp", bufs=2, space="PSUM") as pm:

            feats3 = feats_d[:].rearrange("(b s) d -> b s d", s=SL)

            gixs = cp.tile([128, 16 * R], i16)
            nc.sync.dma_start(out=gixs[:], in_=gidx_d[:])
            iot = cp.tile([128, 8], i16)
            nc.sync.dma_start(out=iot[:], in_=iota_d[:])

            w1t = cp.tile([128, 8, 2, D], f8)
            nc.sync.dma_start(out=w1t[:], in_=w1t_d[:].rearrange("(c two p) d -> p c two d", two=2, p=128))
            w2t = cp.tile([128, 4, 2, D], f8)
            nc.sync.dma_start(out=w2t[:], in_=w2t_d[:].rearrange("(c two p) d -> p c two d", two=2, p=128))
            if not zero_bias:
                b1b = cp.tile([128, D], f32)
                nc.sync.dma_start(out=b1b[:], in_=b1b_d[:])
                b2b = cp.tile([128, D], f32)
                nc.sync.dma_start(out=b2b[:], in_=b2b_d[:])

            def emit_xgather(q):
                # gather+transpose the two 128-row operand sets of round q:
                # xb[p, 8h+c, j] = feats[gidx_q_h[j], 128c+p]
                xb = xbp.tile([128, 16, 128], bf16, tag="xb")
                for h in (0, 1):
                    nc.gpsimd.dma_gather(
                        out_ap=xb[:, 8 * h:8 * h + 8, :],
                        in_ap=feats_d[:],
                        idxs_ap=gixs[:, 16 * q + 8 * h:16 * q + 8 * h + 8],
                        num_idxs=128, num_idxs_reg=128, elem_size=D,
                        transpose=True, queue_num=h)
                return xb

            def emit_xcast(xb):
                xq = xqp.tile([128, 16, 128], f8, tag="xq")
                nc.vector.tensor_copy(out=xq[:, 0:8, :], in_=xb[:, 0:8, :])
                nc.vector.tensor_copy(out=xq[:, 8:16, :], in_=xb[:, 8:16, :])
                return xq

            def emit_fc1(xq):
                # h[(s,b), :] = x @ W1 (x already tanh'd; K=2048, fp8 DR)
                h0 = pm.tile([128, 512], f32, tag="h0")
                h1 = pm.tile([128, 512], f32, tag="h1")
                htt = hp.tile([128, D], bf16, tag="htt")
                hbt = None if zero_bias else hp.tile([128, D], f32, tag="hbt")
                for hn, ht in ((1, h1), (0, h0)):
                    for c in range(8):
                        nc.tensor.matmul(ht[:], xq[:, 2 * c:2 * c + 2, :],
                                         w1t[:, c, :, 512 * hn:512 * (hn + 1)],
                                         start=(c == 0), stop=(c == 7), perf_mode=DR)
                    cs = slice(512 * hn, 512 * (hn + 1))
                    if zero_bias:
                        nc.scalar.activation(out=htt[:, cs], in_=ht[:], func=TANH)
                    else:
                        nc.vector.tensor_tensor(out=hbt[:, cs], in0=ht[:], in1=b1b[:, cs], op=ADD)
                        nc.scalar.activation(out=htt[:, cs], in_=hbt[:, cs], func=TANH)
                return htt

            def emit_hT(htt):
                # SBUF-source gather-transpose: hTb[p, c, j] = htt[j, 128c+p]
                hTb = hp.tile([128, 8, 128], bf16, tag="hTb")
                nc.gpsimd.dma_gather(
                    out_ap=hTb[:], in_ap=htt[:], idxs_ap=iot[:, 0:8],
                    num_idxs=128, num_idxs_reg=128, elem_size=D,
                    transpose=True, queue_num=2,
                    sbuf_tokens_per_rank=128, sbuf_free_dim_per_rank=2 * D)
                hT = hp.tile([128, 8, 128], f8, tag="hT")
                nc.vector.tensor_copy(out=hT[:], in_=hTb[:])
                return hT

            def emit_fc2(hT, r, last):
                f0 = pm.tile([128, 512], f32, tag="f0")
                f1 = pm.tile([128, 512], f32, tag="f1")
                ftb = None if last else fp.tile([128, D], bf16, tag="ftb")
                fbt = None if (last or zero_bias) else fp.tile([128, D], f32, tag="fbt")
                for fn, ft in ((0, f0), (1, f1)):
                    for ci, c in enumerate((2, 3, 0, 1)):
                        nc.tensor.matmul(ft[:], hT[:, 2 * c:2 * c + 2, :],
                                         w2t[:, c, :, 512 * fn:512 * (fn + 1)],
                                         start=(ci == 0), stop=(ci == 3), perf_mode=DR)
                    if not last:
                        cs = slice(512 * fn, 512 * (fn + 1))
                        if zero_bias:
                            nc.scalar.activation(out=ftb[:, cs], in_=ft[:], func=TANH)
                        else:
                            nc.vector.tensor_tensor(out=fbt[:, cs], in0=ft[:], in1=b2b[:, cs], op=ADD)
                            nc.scalar.activation(out=ftb[:, cs], in_=fbt[:, cs], func=TANH)
                if last:
                    # roots are partitions 0:64 of f0/f1; ship the raw f32
                    # root features out -- the (B,1024)@(1024,53) scores +
                    # sigmoid are done on host (0.4% of FLOPs)
                    rootf = fp.tile([64, D], f32, tag="rootf")
                    if zero_bias:
                        nc.vector.tensor_copy(out=rootf[:, 0:512], in_=f0[0:64, :])
                        nc.vector.tensor_copy(out=rootf[:, 512:1024], in_=f1[0:64, :])
                    else:
                        nc.vector.tensor_tensor(out=rootf[:, 0:512], in0=f0[0:64, :], in1=b2b[0:64, 0:512], op=ADD)
                        nc.vector.tensor_tensor(out=rootf[:, 512:1024], in0=f1[0:64, :], in1=b2b[0:64, 512:1024], op=ADD)
                    nc.sync.dma_start(out=out_d[:], in_=rootf[:])
                else:
                    for s in range(2):
                        nc.sync.dma_start(out=feats3[:, 64 + 2 * r + s, :],
                                          in_=ftb[64 * s:64 * (s + 1), :])

            # ---- software-pipelined main loop ----
            # PE order: FC1_0, FC1_1, FC2_0, FC1_2, FC2_1, ..., FC2_{R-1}.
            # Round q's gather is emitted right after scatter_{q-3}, so it
            # waits only on scatters <= q-3 (the packer guarantees children
            # <= q-DIST).
            xb = {q: emit_xgather(q) for q in range(min(DIST, R))}
            xq = {0: emit_xcast(xb.pop(0))}
            if 1 < R:
                xq[1] = emit_xcast(xb.pop(1))
            htt = {0: emit_fc1(xq.pop(0))}
            for r in range(R):
                hT = emit_hT(htt.pop(r))
                if r + 1 < R:
                    htt[r + 1] = emit_fc1(xq.pop(r + 1))
                if r + 2 < R:
                    xq[r + 2] = emit_xcast(xb.pop(r + 2))
                emit_fc2(hT, r, last=(r == R - 1))
                if r + DIST < R:
                    xb[r + DIST] = emit_xgather(r + DIST)

    nc.compile()
    _PROG[key] = nc
    return nc


def _node_weight_like_reference(rep, n_per_bag):
    """Bit-faithful mirror of the reference's eager node_weight computation
    (reference runs on CPU jax; mirror that exactly)."""
    import jax
    import jax.numpy as jnp
    cpu = jax.local_devices(backend="cpu")[0]
    with jax.default_device(cpu):
        d = rep.shape[-1]
        bags = jnp.asarray(np.ascontiguousarray(rep, dtype=np.float32)).reshape(-1, n_per_bag, d)
        norms = jnp.linalg.norm(bags, axis=-1)
        gram = jnp.einsum('bnd,bmd->bnm', bags, bags)
        sims = gram / jnp.maximum(norms[:, :, None] * norms[:, None, :], 1e-8)
        node_distance = sims.sum(axis=1)
        node_weight = jax.nn.softmax(node_distance, axis=-1)
        return np.asarray(node_weight).astype(np.float32)


def _huffman_schedule(w):
    """Replay the reference scan's weight bookkeeping (exact f32) and emit
    per-bag merge operand slots: leaves 0..63, merge t -> 64+t."""
    B, n = w.shape
    wref = w.copy()
    alive = np.ones((B, n), bool)
    prov = np.tile(np.arange(n, dtype=np.int64), (B, 1))
    ar = np.arange(B)
    gl = np.zeros((B, n - 1), np.int64)
    gr = np.zeros((B, n - 1), np.int64)
    INF = np.float32(np.inf)
    for t in range(n - 1):
        wm = np.where(alive, wref, INF)
        i1 = np.argmin(wm, axis=1)
        wm2 = wm.copy()
        wm2[ar, i1] = INF
        i2 = np.argmin(wm2, axis=1)
        gl[:, t] = prov[ar, i1]
        gr[:, t] = prov[ar, i2]
        wref[ar, i1] = wm[ar, i1] + wm[ar, i2]
        alive[ar, i2] = False
        prov[ar, i1] = n + t
    return gl, gr


def _pack_rounds(gl, gr, n=NN, dist=DIST):
    """List-schedule each bag's n-1 merges into pair-rounds (2 independent
    merges per round; children must be done <= r-dist; priority = longest
    path to root).  The root merge is then pinned to (last real round + 1,
    slot 0) for every bag, so the device reads all roots from the final
    round's PSUM result tile.  Returns (rounds_of, slot_of, R)."""
    B, m = gl.shape
    rounds_of = np.zeros((B, m), np.int64)
    slot_of = np.zeros((B, m), np.int64)
    last_nonroot = 0
    root_child_max = 0
    for b in range(B):
        cl, cr = gl[b], gr[b]
        parents = np.full(m, -1, np.int64)
        ndep = np.zeros(m, np.int32)
        for j in range(m):
            for s in (cl[j], cr[j]):
                if s >= n:
                    ndep[j] += 1
                    parents[s - n] = j
        height = np.zeros(m, np.int64)
        for j in range(m - 1, -1, -1):
            p = parents[j]
            if p >= 0:
                height[j] = height[p] + 1
        done = np.full(m, 10**9, np.int64)
        remaining = ndep.copy()
        scheduled = 0
        r = 0
        while scheduled < m:
            ready = [j for j in range(m)
                     if remaining[j] == 0 and done[j] == 10**9
                     and all((s < n or done[s - n] <= r - dist) for s in (cl[j], cr[j]))]
            ready.sort(key=lambda j: (-height[j], j))
            for s_idx, j in enumerate(ready[:2]):
                rounds_of[b, j] = r
                slot_of[b, j] = s_idx
                done[j] = r
                scheduled += 1
                p = parents[j]
                if p >= 0:
                    remaining[p] -= 1
            r += 1
            assert r < 8 * m, "packer stuck"
        last_nonroot = max(last_nonroot, rounds_of[b, :m - 1].max())
        for s in (cl[m - 1], cr[m - 1]):
            if s >= n:
                root_child_max = max(root_child_max, int(rounds_of[b, s - n]))
    # root round: after every non-root merge AND >= dist past every root
    # child so the root round's gather needs no special casing
    root_round = max(last_nonroot + 1, root_child_max + dist)
    rounds_of[:, m - 1] = root_round
    slot_of[:, m - 1] = 0
    R = root_round + 1
    # verify the dist invariant (the device pipeline depends on it)
    for b in range(B):
        for j in range(m):
            r = rounds_of[b, j]
            for s in (gl[b, j], gr[b, j]):
                if s >= n:
                    assert rounds_of[b, s - n] <= r - dist, \
                        f"dist-{dist} violated: bag {b} merge {j}"
    return rounds_of, slot_of, R


def _wrap16(arr):
    """Pack a flat int array of 128 gather positions into the SWDGE
    wrapped-16 idx layout [128, 8]: position n -> [n%16, n//16], replicated
    across the 8 partition groups."""
    block = arr.astype(np.int16).reshape(8, 16).T  # [16, 8]
    return np.tile(block, (8, 1))  # [128, 8]


def _prepare(rep, fc1_w, fc1_b, fc2_w, fc2_b, rel_emb, n_per_bag, **kw):
    n_per_bag = int(n_per_bag)
    assert n_per_bag == NN and rep.shape[-1] == D
    rep = np.ascontiguousarray(rep, dtype=np.float32)

    w = _node_weight_like_reference(rep, n_per_bag)
    gl, gr = _huffman_schedule(w)
    rounds_of, slot_of, R = _pack_rounds(gl, gr)
    SL = 64 + 2 * R
    zb = (not np.any(np.asarray(fc1_b))) and (not np.any(np.asarray(fc2_b)))

    nc = _build_program(R, SL, zb)

    import ml_dtypes
    f8 = ml_dtypes.float8_e4m3fn
    w1t = np.ascontiguousarray(np.asarray(fc1_w, np.float32).T).astype(f8)   # (2D, D)
    w2t = np.ascontiguousarray(np.asarray(fc2_w, np.float32).T).astype(f8)   # (D, D)
    b1b = np.ascontiguousarray(np.broadcast_to(np.asarray(fc1_b, np.float32), (128, D)))
    b2b = np.ascontiguousarray(np.broadcast_to(np.asarray(fc2_b, np.float32), (128, D)))
    iota = _wrap16(np.arange(128))

    merge_slot = 64 + 2 * rounds_of + slot_of          # (B, 63)
    B = gl.shape[0]
    m = gl.shape[1]

    in_maps = []
    for c in range(NCORES):
        b0 = c * NB
        gidx = np.zeros((128, 16 * R), np.int16)
        for q in range(R):
            for h in (0, 1):
                child = gl if h == 0 else gr
                arr = np.zeros(128, np.int64)
                for lb in range(NB):
                    arr[lb] = lb * SL          # slot-0 pad: a leaf row
                    arr[64 + lb] = lb * SL     # slot-1 pad
                for lb in range(NB):
                    b = b0 + lb
                    js = np.where(rounds_of[b] == q)[0]
                    for j in js:
                        s = slot_of[b, j]
                        ch = child[b, j]
                        row = ch if ch < NN else merge_slot[b, ch - NN]
                        arr[s * NB + lb] = lb * SL + row
                gidx[:, 16 * q + 8 * h:16 * q + 8 * h + 8] = _wrap16(arr)

        feats = np.zeros((NB * SL, D), ml_dtypes.bfloat16)
        leaves = np.tanh(rep[b0 * NN:(b0 + NB) * NN].reshape(NB, NN, D)).astype(ml_dtypes.bfloat16)
        feats.reshape(NB, SL, D)[:, :NN, :] = leaves
        in_maps.append({
            "feats": feats,
            "w1t": w1t, "w2t": w2t,
            "b1b": b1b, "b2b": b2b, "gidx": gidx, "iota": iota,
        })
    return nc, in_maps


def kernel(rep, fc1_w, fc1_b, fc2_w, fc2_b, rel_emb, n_per_bag, **kw):
    nc, in_maps = _prepare(rep, fc1_w, fc1_b, fc2_w, fc2_b, rel_emb, n_per_bag)
    from concourse import bass_utils
    res = bass_utils.run_bass_kernel_spmd(nc, in_maps, core_ids=list(range(NCORES)))
    root = np.concatenate([res.results[c]["out"] for c in range(NCORES)], axis=0)
    scores = root.astype(np.float32) @ np.asarray(rel_emb, np.float32).T
    out = 1.0 / (1.0 + np.exp(-scores, dtype=np.float64))
    return np.ascontiguousarray(out.astype(np.float32))


# revision 9
# speedup vs baseline: 1.4639x; 1.4639x over previous
"""Trainium2 Bass kernel for nn_ModelRQuery_5806795784426.

Strategy (data-parallel over bags, 8 cores x 64 bags):
  - node_weight (cosine-sim softmax) is computed with the exact same eager
    jax ops as the reference, so the Huffman merge schedule derived from it
    is bit-faithful to the reference's argmin decisions on this backend.
  - The Huffman weight evolution is replayed on host (pure IEEE f32 adds on
    identical bits -> identical schedule), producing per-bag merge pairs.
  - Per bag, the 63 merges are list-scheduled into R pair-rounds (2
    independent merges per round -> M=128 rows, full PE array) with
    children >= 3 rounds earlier (dist-3), so every round's gather is
    covered by a full round of PE work and the PE never waits on the
    scatter->gather chain.  Each bag's root merge is pinned to the final
    round, slot 0.
  - tanh is applied at PRODUCTION: DRAM feats rows hold tanh'd bf16
    features (leaves host-pre-tanh'd).
  - v2: ALL transposes are off the PE.  The gather is dma_gather
    (transpose=True): it both gathers the 2x128 child rows by index and
    writes them transposed (feature-major) into SBUF in one SWDGE
    instruction.  The FC1->FC2 h-transpose uses the SBUF-source variant of
    the same instruction.  The PE runs ONLY the FC1/FC2 fp8 DoubleRow
    matmuls, software-pipelined as [..., FC1_{r+1}, FC2_r, ...] so the
    tanh/transpose/gather latency of round r hides under FC1_{r+1}.
  - Final: raw f32 root features are DMA'd out; the tiny scores matmul
    (B,1024)@(1024,53) + sigmoid run on host.
"""

import numpy as np

NB = 64      # bags per core
NN = 64      # nodes (leaves) per bag
D = 1024
NSTEP = NN - 1
NCORES = 8
DIST = 3     # min round separation between child production and consumption

_PROG = {}


def _build_program(R, SL, zero_bias):
    key = (R, SL, zero_bias, "v3")
    if key in _PROG:
        return _PROG[key]
    import concourse.bass as bass
    import concourse.bacc as bacc
    import concourse.tile as tile

    mybir = bass.mybir
    f32 = mybir.dt.float32
    bf16 = mybir.dt.bfloat16
    f8 = mybir.dt.float8e4
    i16 = mybir.dt.int16
    TANH = mybir.ActivationFunctionType.Tanh
    COPY = mybir.ActivationFunctionType.Copy
    ADD = mybir.AluOpType.add
    DR = mybir.MatmulPerfMode.DoubleRow

    nc = bacc.Bacc(None, target_bir_lowering=False, num_swdge_queues=2)
    # feats rows: bag*SL + slot, tanh'd bf16; slots 0..63 = leaves
    # (host-prefilled tanh(rep)), slot 64+2r+s = tanh(output) of round r
    # in-round slot s.
    feats_d = nc.dram_tensor("feats", [NB * SL, D], bf16, kind="ExternalInput")
    w1t_d = nc.dram_tensor("w1t", [2 * D, D], f8, kind="ExternalInput")
    w2t_d = nc.dram_tensor("w2t", [D, D], f8, kind="ExternalInput")
    b1b_d = nc.dram_tensor("b1b", [128, D], f32, kind="ExternalInput")
    b2b_d = nc.dram_tensor("b2b", [128, D], f32, kind="ExternalInput")
    # int16 gather indices: for round q, op h, the 128 gathered rows are at
    # gidx[:, 16q+8h : 16q+8h+8] in the SWDGE "wrapped-16" layout
    # (position n -> [n%16, n//16], replicated across partition groups).
    gidx_d = nc.dram_tensor("gidx", [128, 16 * R], i16, kind="ExternalInput")
    ident_d = nc.dram_tensor("ident", [128, 128], bf16, kind="ExternalInput")
    out_d = nc.dram_tensor("out", [NB, D], f32, kind="ExternalOutput")

    with tile.TileContext(nc) as tc:
        with tc.tile_pool(name="const", bufs=1) as cp, \
             tc.tile_pool(name="xb", bufs=3) as xbp, \
             tc.tile_pool(name="xq", bufs=2) as xqp, \
             tc.tile_pool(name="hp", bufs=2) as hp, \
             tc.tile_pool(name="fp", bufs=2) as fp, \
             tc.tile_pool(name="tpp", bufs=2, space="PSUM") as pt, \
             tc.tile_pool(name="mmp", bufs=1, space="PSUM") as pmf, \
             tc.tile_pool(name="mmh", bufs=2, space="PSUM") as pmh:

            feats3 = feats_d[:].rearrange("(b s) d -> b s d", s=SL)

            gixs = cp.tile([128, 16 * R], i16)
            nc.sync.dma_start(out=gixs[:], in_=gidx_d[:])
            ident = cp.tile([128, 128], bf16)
            nc.sync.dma_start(out=ident[:], in_=ident_d[:])

            w1t = cp.tile([128, 8, 2, D], f8)
            nc.sync.dma_start(out=w1t[:], in_=w1t_d[:].rearrange("(c two p) d -> p c two d", two=2, p=128))
            w2t = cp.tile([128, 4, 2, D], f8)
            nc.sync.dma_start(out=w2t[:], in_=w2t_d[:].rearrange("(c two p) d -> p c two d", two=2, p=128))
            if not zero_bias:
                b1b = cp.tile([128, D], f32)
                nc.sync.dma_start(out=b1b[:], in_=b1b_d[:])
                b2b = cp.tile([128, D], f32)
                nc.sync.dma_start(out=b2b[:], in_=b2b_d[:])

            def emit_xgather(q):
                # gather+transpose the two 128-row operand sets of round q:
                # xb[p, 8h+c, j] = feats[gidx_q_h[j], 128c+p]
                xb = xbp.tile([128, 16, 128], bf16, tag="xb")
                for h in (0, 1):
                    nc.gpsimd.dma_gather(
                        out_ap=xb[:, 8 * h:8 * h + 8, :],
                        in_ap=feats_d[:],
                        idxs_ap=gixs[:, 16 * q + 8 * h:16 * q + 8 * h + 8],
                        num_idxs=128, num_idxs_reg=128, elem_size=D,
                        transpose=True, queue_num=h)
                return xb

            def emit_xcast(xb):
                # bf16 gathered-transposed operands -> fp8 lhsT; op0 on the
                # scalar engine, op1 on DVE so both halves land in parallel
                xq = xqp.tile([128, 16, 128], f8, tag="xq")
                nc.scalar.activation(out=xq[:, 0:8, :], in_=xb[:, 0:8, :], func=COPY)
                nc.vector.tensor_copy(out=xq[:, 8:16, :], in_=xb[:, 8:16, :])
                return xq

            def emit_fc1(xq):
                # h[(s,b), :] = x @ W1 (x already tanh'd; K=2048, fp8 DR)
                h0 = pmh.tile([128, 512], f32, tag="h0")
                h1 = pmh.tile([128, 512], f32, tag="h1")
                htt = hp.tile([128, D], bf16, tag="htt")
                hbt = None if zero_bias else hp.tile([128, D], f32, tag="hbt")
                for hn, ht in ((1, h1), (0, h0)):
                    for c in range(8):
                        nc.tensor.matmul(ht[:], xq[:, 2 * c:2 * c + 2, :],
                                         w1t[:, c, :, 512 * hn:512 * (hn + 1)],
                                         start=(c == 0), stop=(c == 7), perf_mode=DR)
                    cs = slice(512 * hn, 512 * (hn + 1))
                    if zero_bias:
                        nc.scalar.activation(out=htt[:, cs], in_=ht[:], func=TANH)
                    else:
                        nc.vector.tensor_tensor(out=hbt[:, cs], in0=ht[:], in1=b1b[:, cs], op=ADD)
                        nc.scalar.activation(out=htt[:, cs], in_=hbt[:, cs], func=TANH)
                return htt

            def emit_hT(htt):
                # 8 PE transposes (128x128 bf16) + 2 DVE cast-copies
                hT = hp.tile([128, 8, 128], f8, tag="hT")
                for q in (1, 0):
                    ps = pt.tile([128, 4, 128], bf16, tag="htp")
                    for j in range(4):
                        c = 4 * q + j
                        nc.tensor.transpose(out=ps[:, j, :], in_=htt[:, 128 * c:128 * (c + 1)],
                                            identity=ident[:])
                    nc.vector.tensor_copy(out=hT[:, 4 * q:4 * q + 4, :], in_=ps[:])
                return hT

            def emit_fc2(hT, r, last):
                f0 = pmf.tile([128, 512], f32, tag="f0")
                f1 = pmf.tile([128, 512], f32, tag="f1")
                ftb = None if last else fp.tile([128, D], bf16, tag="ftb")
                fbt = None if (last or zero_bias) else fp.tile([128, D], f32, tag="fbt")
                for fn, ft in ((0, f0), (1, f1)):
                    for ci, c in enumerate((2, 3, 0, 1)):
                        nc.tensor.matmul(ft[:], hT[:, 2 * c:2 * c + 2, :],
                                         w2t[:, c, :, 512 * fn:512 * (fn + 1)],
                                         start=(ci == 0), stop=(ci == 3), perf_mode=DR)
                    if not last:
                        cs = slice(512 * fn, 512 * (fn + 1))
                        if zero_bias:
                            nc.scalar.activation(out=ftb[:, cs], in_=ft[:], func=TANH)
                        else:
                            nc.vector.tensor_tensor(out=fbt[:, cs], in0=ft[:], in1=b2b[:, cs], op=ADD)
                            nc.scalar.activation(out=ftb[:, cs], in_=fbt[:, cs], func=TANH)
                if last:
                    # roots are partitions 0:64 of f0/f1; ship the raw f32
                    # root features out -- the (B,1024)@(1024,53) scores +
                    # sigmoid are done on host (0.4% of FLOPs)
                    rootf = fp.tile([64, D], f32, tag="rootf")
                    if zero_bias:
                        nc.vector.tensor_copy(out=rootf[:, 0:512], in_=f0[0:64, :])
                        nc.vector.tensor_copy(out=rootf[:, 512:1024], in_=f1[0:64, :])
                    else:
                        nc.vector.tensor_tensor(out=rootf[:, 0:512], in0=f0[0:64, :], in1=b2b[0:64, 0:512], op=ADD)
                        nc.vector.tensor_tensor(out=rootf[:, 512:1024], in0=f1[0:64, :], in1=b2b[0:64, 512:1024], op=ADD)
                    nc.sync.dma_start(out=out_d[:], in_=rootf[:])
                else:
                    for s in range(2):
                        nc.sync.dma_start(out=feats3[:, 64 + 2 * r + s, :],
                                          in_=ftb[64 * s:64 * (s + 1), :])

            # ---- software-pipelined main loop ----
            # PE order: FC1_0, FC1_1, then per round r:
            #   [T(h_r) x8, FC2_r, FC1_{r+2}]
            # so the tanh-h/transpose chain of round r hides under FC1_{r+1}
            # (previous iteration) and the scatter_r -> gather_{r+3} ->
            # cast_{r+3} chain hides under ~1.5 rounds of PE work (the
            # packer guarantees children <= r-DIST, DIST=3).
            xb = {q: emit_xgather(q) for q in range(min(DIST, R))}
            xq = {0: emit_xcast(xb.pop(0))}
            if 1 < R:
                xq[1] = emit_xcast(xb.pop(1))
            htt = {0: emit_fc1(xq.pop(0))}
            if 1 < R:
                htt[1] = emit_fc1(xq.pop(1))
            for r in range(R):
                hT = emit_hT(htt.pop(r))
                if r + 2 < R:
                    xq[r + 2] = emit_xcast(xb.pop(r + 2))
                emit_fc2(hT, r, last=(r == R - 1))
                if r + 2 < R:
                    htt[r + 2] = emit_fc1(xq.pop(r + 2))
                if r + DIST < R:
                    xb[r + DIST] = emit_xgather(r + DIST)

    nc.compile()
    _PROG[key] = nc
    return nc


def _node_weight_like_reference(rep, n_per_bag):
    """Bit-faithful mirror of the reference's eager node_weight computation
    (reference runs on CPU jax; mirror that exactly)."""
    import jax
    import jax.numpy as jnp
    cpu = jax.local_devices(backend="cpu")[0]
    with jax.default_device(cpu):
        d = rep.shape[-1]
        bags = jnp.asarray(np.ascontiguousarray(rep, dtype=np.float32)).reshape(-1, n_per_bag, d)
        norms = jnp.linalg.norm(bags, axis=-1)
        gram = jnp.einsum('bnd,bmd->bnm', bags, bags)
        sims = gram / jnp.maximum(norms[:, :, None] * norms[:, None, :], 1e-8)
        node_distance = sims.sum(axis=1)
        node_weight = jax.nn.softmax(node_distance, axis=-1)
        return np.asarray(node_weight).astype(np.float32)


def _huffman_schedule(w):
    """Replay the reference scan's weight bookkeeping (exact f32) and emit
    per-bag merge operand slots: leaves 0..63, merge t -> 64+t."""
    B, n = w.shape
    wref = w.copy()
    alive = np.ones((B, n), bool)
    prov = np.tile(np.arange(n, dtype=np.int64), (B, 1))
    ar = np.arange(B)
    gl = np.zeros((B, n - 1), np.int64)
    gr = np.zeros((B, n - 1), np.int64)
    INF = np.float32(np.inf)
    for t in range(n - 1):
        wm = np.where(alive, wref, INF)
        i1 = np.argmin(wm, axis=1)
        wm2 = wm.copy()
        wm2[ar, i1] = INF
        i2 = np.argmin(wm2, axis=1)
        gl[:, t] = prov[ar, i1]
        gr[:, t] = prov[ar, i2]
        wref[ar, i1] = wm[ar, i1] + wm[ar, i2]
        alive[ar, i2] = False
        prov[ar, i1] = n + t
    return gl, gr


def _pack_rounds(gl, gr, n=NN, dist=DIST):
    """List-schedule each bag's n-1 merges into pair-rounds (2 independent
    merges per round; children must be done <= r-dist; priority = longest
    path to root).  The root merge is then pinned to (last real round + 1,
    slot 0) for every bag, so the device reads all roots from the final
    round's PSUM result tile.  Returns (rounds_of, slot_of, R)."""
    B, m = gl.shape
    rounds_of = np.zeros((B, m), np.int64)
    slot_of = np.zeros((B, m), np.int64)
    last_nonroot = 0
    root_child_max = 0
    for b in range(B):
        cl, cr = gl[b], gr[b]
        parents = np.full(m, -1, np.int64)
        ndep = np.zeros(m, np.int32)
        for j in range(m):
            for s in (cl[j], cr[j]):
                if s >= n:
                    ndep[j] += 1
                    parents[s - n] = j
        height = np.zeros(m, np.int64)
        for j in range(m - 1, -1, -1):
            p = parents[j]
            if p >= 0:
                height[j] = height[p] + 1
        done = np.full(m, 10**9, np.int64)
        remaining = ndep.copy()
        scheduled = 0
        r = 0
        while scheduled < m:
            ready = [j for j in range(m)
                     if remaining[j] == 0 and done[j] == 10**9
                     and all((s < n or done[s - n] <= r - dist) for s in (cl[j], cr[j]))]
            ready.sort(key=lambda j: (-height[j], j))
            for s_idx, j in enumerate(ready[:2]):
                rounds_of[b, j] = r
                slot_of[b, j] = s_idx
                done[j] = r
                scheduled += 1
                p = parents[j]
                if p >= 0:
                    remaining[p] -= 1
            r += 1
            assert r < 8 * m, "packer stuck"
        last_nonroot = max(last_nonroot, rounds_of[b, :m - 1].max())
        for s in (cl[m - 1], cr[m - 1]):
            if s >= n:
                root_child_max = max(root_child_max, int(rounds_of[b, s - n]))
    # root round: after every non-root merge AND >= dist past every root
    # child so the root round's gather needs no special casing
    root_round = max(last_nonroot + 1, root_child_max + dist)
    rounds_of[:, m - 1] = root_round
    slot_of[:, m - 1] = 0
    R = root_round + 1
    # verify the dist invariant (the device pipeline depends on it)
    for b in range(B):
        for j in range(m):
            r = rounds_of[b, j]
            for s in (gl[b, j], gr[b, j]):
                if s >= n:
                    assert rounds_of[b, s - n] <= r - dist, \
                        f"dist-{dist} violated: bag {b} merge {j}"
    return rounds_of, slot_of, R


def _wrap16(arr):
    """Pack a flat int array of 128 gather positions into the SWDGE
    wrapped-16 idx layout [128, 8]: position n -> [n%16, n//16], replicated
    across the 8 partition groups."""
    block = arr.astype(np.int16).reshape(8, 16).T  # [16, 8]
    return np.tile(block, (8, 1))  # [128, 8]


def _prepare(rep, fc1_w, fc1_b, fc2_w, fc2_b, rel_emb, n_per_bag, **kw):
    n_per_bag = int(n_per_bag)
    assert n_per_bag == NN and rep.shape[-1] == D
    rep = np.ascontiguousarray(rep, dtype=np.float32)

    w = _node_weight_like_reference(rep, n_per_bag)
    gl, gr = _huffman_schedule(w)
    rounds_of, slot_of, R = _pack_rounds(gl, gr)
    SL = 64 + 2 * R
    zb = (not np.any(np.asarray(fc1_b))) and (not np.any(np.asarray(fc2_b)))

    nc = _build_program(R, SL, zb)

    import ml_dtypes
    f8 = ml_dtypes.float8_e4m3fn
    w1t = np.ascontiguousarray(np.asarray(fc1_w, np.float32).T).astype(f8)   # (2D, D)
    w2t = np.ascontiguousarray(np.asarray(fc2_w, np.float32).T).astype(f8)   # (D, D)
    b1b = np.ascontiguousarray(np.broadcast_to(np.asarray(fc1_b, np.float32), (128, D)))
    b2b = np.ascontiguousarray(np.broadcast_to(np.asarray(fc2_b, np.float32), (128, D)))
    ident = np.eye(128, dtype=ml_dtypes.bfloat16)

    merge_slot = 64 + 2 * rounds_of + slot_of          # (B, 63)
    B = gl.shape[0]
    m = gl.shape[1]

    in_maps = []
    for c in range(NCORES):
        b0 = c * NB
        gidx = np.zeros((128, 16 * R), np.int16)
        for q in range(R):
            for h in (0, 1):
                child = gl if h == 0 else gr
                arr = np.zeros(128, np.int64)
                for lb in range(NB):
                    arr[lb] = lb * SL          # slot-0 pad: a leaf row
                    arr[64 + lb] = lb * SL     # slot-1 pad
                for lb in range(NB):
                    b = b0 + lb
                    js = np.where(rounds_of[b] == q)[0]
                    for j in js:
                        s = slot_of[b, j]
                        ch = child[b, j]
                        row = ch if ch < NN else merge_slot[b, ch - NN]
                        arr[s * NB + lb] = lb * SL + row
                gidx[:, 16 * q + 8 * h:16 * q + 8 * h + 8] = _wrap16(arr)

        feats = np.zeros((NB * SL, D), ml_dtypes.bfloat16)
        leaves = np.tanh(rep[b0 * NN:(b0 + NB) * NN].reshape(NB, NN, D)).astype(ml_dtypes.bfloat16)
        feats.reshape(NB, SL, D)[:, :NN, :] = leaves
        in_maps.append({
            "feats": feats,
            "w1t": w1t, "w2t": w2t,
            "b1b": b1b, "b2b": b2b, "gidx": gidx, "ident": ident,
        })
    return nc, in_maps


def kernel(rep, fc1_w, fc1_b, fc2_w, fc2_b, rel_emb, n_per_bag, **kw):
    nc, in_maps = _prepare(rep, fc1_w, fc1_b, fc2_w, fc2_b, rel_emb, n_per_bag)
    from concourse import bass_utils
    res = bass_utils.run_bass_kernel_spmd(nc, in_maps, core_ids=list(range(NCORES)))
    root = np.concatenate([res.results[c]["out"] for c in range(NCORES)], axis=0)
    scores = root.astype(np.float32) @ np.asarray(rel_emb, np.float32).T
    out = 1.0 / (1.0 + np.exp(-scores, dtype=np.float64))
    return np.ascontiguousarray(out.astype(np.float32))
